# revision 2
# baseline (speedup 1.0000x reference)
"""Dice-loss kernel for Trainium2 (Bass/Tile), 8-way data parallel.

Math (per stage s, batch b, organ o):
    inter[s,b,o] = sum_v pred[s][b,o+1,v] * (target[b,v] == o+1)
    p2[s,b,o]    = sum_v pred[s][b,o+1,v]^2
    t2[b,o]      = sum_v (target[b,v] == o+1)
    dice[b]      = sum_{s,o} 2*inter/(p2+t2+eps) / 13
    loss         = mean_b (2 - dice[b])

Device strategy: the flat spatial volume (48*256*256 voxels) is split
contiguously 8 ways; each core streams its 1/8 of both preds (channels
1..13 only) and of the target.  Per (b,o): one DVE tensor_scalar
(is_equal, accum_out) builds the one-hot mask and its count t2.  Per
(s,b,o): one DVE tensor_tensor(mult) forms the masked product (bf16),
which the TensorEngine reduces with ones-vector matmuls accumulated in a
PSUM row, extracted by a ScalarE Copy+accum; p^2 is one ScalarE
activation(Square, accum_out).  Per-partition partials are DMAed out;
the final tiny reduction and dice formula run on host in float64.
"""

import numpy as np

import concourse.bacc as bacc
import concourse.tile as tile
from concourse import mybir
from concourse.bass_utils import run_bass_kernel_spmd

N_CORES = 8
B = 2
NUM_ORGAN = 13
VOX = 48 * 256 * 256          # 3,145,728 voxels per (b, organ) volume
SHARD = VOX // N_CORES        # 393,216 voxels per core
P = 128                       # SBUF partitions
FD = SHARD // P               # 3072 free-dim elements per partition
MM_N = 512                    # matmul free-dim chunk (one PSUM bank)
EPS = 1e-05

_NC_CACHE = {}


def build_nc():
    f32 = mybir.dt.float32
    bf16 = mybir.dt.bfloat16
    nc = bacc.Bacc(
        "TRN2",
        target_bir_lowering=False,
        debug=False,
        num_devices=N_CORES,
    )
    p1 = nc.dram_tensor("p1", [B, NUM_ORGAN, P, FD], f32, kind="ExternalInput").ap()
    p2 = nc.dram_tensor("p2", [B, NUM_ORGAN, P, FD], f32, kind="ExternalInput").ap()
    tt = nc.dram_tensor("tt", [B, P, FD], f32, kind="ExternalInput").ap()
    out_t2 = nc.dram_tensor(
        "out_t2", [P, B * NUM_ORGAN], f32, kind="ExternalOutput"
    ).ap()
    out_inter = nc.dram_tensor(
        "out_inter", [1, 2 * B * NUM_ORGAN], f32, kind="ExternalOutput"
    ).ap()
    out_p2 = nc.dram_tensor(
        "out_p2", [P, 2 * B * NUM_ORGAN], f32, kind="ExternalOutput"
    ).ap()

    with tile.TileContext(nc) as tc, \
            tc.tile_pool(name="pin", bufs=4) as pin_pool, \
            tc.tile_pool(name="tin", bufs=2) as t_pool, \
            tc.tile_pool(name="mask", bufs=2) as mask_pool, \
            tc.tile_pool(name="prod", bufs=2) as prod_pool, \
            tc.tile_pool(name="scr", bufs=2) as scr_pool, \
            tc.tile_pool(name="psum", bufs=6, space="PSUM") as psum_pool, \
            tc.tile_pool(name="acc", bufs=1) as acc_pool:
        acc_t2 = acc_pool.tile([P, B * NUM_ORGAN], f32, tag="acc_t2")
        acc_in = acc_pool.tile([1, 2 * B * NUM_ORGAN], f32, tag="acc_in")
        acc_p2 = acc_pool.tile([P, 2 * B * NUM_ORGAN], f32, tag="acc_p2")
        ones = acc_pool.tile([P, 1], bf16, tag="ones")
        nc.vector.memset(ones[:], 1.0)
        for b in range(B):
            t_tile = t_pool.tile([P, FD], f32, tag="t")
            nc.sync.dma_start(t_tile[:], tt[b])
            for o in range(NUM_ORGAN):
                col = b * NUM_ORGAN + o
                mask = mask_pool.tile([P, FD], f32, tag="m")
                nc.vector.tensor_scalar(
                    mask[:],
                    t_tile[:],
                    float(o + 1),
                    None,
                    op0=mybir.AluOpType.is_equal,
                    op1=mybir.AluOpType.add,
                    accum_out=acc_t2[:, col:col + 1],
                )
                for s, pin in enumerate((p1, p2)):
                    col2 = col * 2 + s
                    pt = pin_pool.tile([P, FD], f32, tag="p")
                    nc.sync.dma_start(pt[:], pin[b, o])
                    # p^2 partial sums on ScalarE
                    scr_a = scr_pool.tile([P, FD], bf16, tag="sa")
                    nc.scalar.activation(
                        out=scr_a[:],
                        in_=pt[:],
                        func=mybir.ActivationFunctionType.Square,
                        accum_out=acc_p2[:, col2:col2 + 1],
                    )
                    # masked product on DVE (bf16 out), reduced on PE
                    prod = prod_pool.tile([P, FD], bf16, tag="pr")
                    nc.vector.tensor_tensor(
                        prod[:], pt[:], mask[:], op=mybir.AluOpType.mult
                    )
                    ps = psum_pool.tile([1, MM_N], f32, tag="ps")
                    nchunk = FD // MM_N
                    for j in range(nchunk):
                        nc.tensor.matmul(
                            ps[:],
                            ones[:],
                            prod[:, j * MM_N:(j + 1) * MM_N],
                            start=(j == 0),
                            stop=(j == nchunk - 1),
                        )
                    scr_e = scr_pool.tile([1, MM_N], f32, tag="se")
                    nc.scalar.activation(
                        out=scr_e[:],
                        in_=ps[:],
                        func=mybir.ActivationFunctionType.Copy,
                        accum_out=acc_in[:, col2:col2 + 1],
                    )
        nc.sync.dma_start(out_t2[:], acc_t2[:])
        nc.sync.dma_start(out_inter[:], acc_in[:])
        nc.sync.dma_start(out_p2[:], acc_p2[:])
    nc.compile()
    return nc


def _get_nc():
    if "nc" not in _NC_CACHE:
        _NC_CACHE["nc"] = build_nc()
    return _NC_CACHE["nc"]


def make_in_maps(pred_stage1, pred_stage2, target):
    """Shard full inputs into per-core input maps (host-side)."""
    p1 = np.asarray(pred_stage1)[:, 1:1 + NUM_ORGAN].reshape(B, NUM_ORGAN, VOX)
    p2 = np.asarray(pred_stage2)[:, 1:1 + NUM_ORGAN].reshape(B, NUM_ORGAN, VOX)
    tf = np.asarray(target).astype(np.float32).reshape(B, VOX)
    in_maps = []
    for c in range(N_CORES):
        sl = slice(c * SHARD, (c + 1) * SHARD)
        in_maps.append({
            "p1": np.ascontiguousarray(p1[:, :, sl]).reshape(B, NUM_ORGAN, P, FD),
            "p2": np.ascontiguousarray(p2[:, :, sl]).reshape(B, NUM_ORGAN, P, FD),
            "tt": np.ascontiguousarray(tf[:, sl]).reshape(B, P, FD),
        })
    return in_maps


def finalize(results):
    """Combine per-core partials into the scalar loss (host-side)."""
    t2 = np.zeros(B * NUM_ORGAN, np.float64)
    inter = np.zeros(2 * B * NUM_ORGAN, np.float64)
    p2s = np.zeros(2 * B * NUM_ORGAN, np.float64)
    for r in results:
        t2 += r["out_t2"].astype(np.float64).sum(axis=0)
        inter += r["out_inter"].astype(np.float64).sum(axis=0)
        p2s += r["out_p2"].astype(np.float64).sum(axis=0)
    t2 = t2.reshape(B, NUM_ORGAN)
    inter = inter.reshape(B, NUM_ORGAN, 2)
    p2s = p2s.reshape(B, NUM_ORGAN, 2)
    dice = 2.0 * inter / (p2s + t2[:, :, None] + EPS)   # (B, 13, 2)
    dice_b = dice.sum(axis=(1, 2)) / NUM_ORGAN          # (B,)
    loss = np.mean(2.0 - dice_b)
    return np.array(loss, dtype=np.float32)


def kernel(pred_stage1, pred_stage2, target):
    nc = _get_nc()
    in_maps = make_in_maps(pred_stage1, pred_stage2, target)
    res = run_bass_kernel_spmd(nc, in_maps, core_ids=list(range(N_CORES)))
    return finalize(res.results)


# revision 7
# speedup vs baseline: 3.9800x; 3.9800x over previous
"""Dice-loss kernel for Trainium2 (Bass/Tile), 8-way data parallel.

Math (per stage s, batch b, organ o):
    inter[s,b,o] = sum_v pred[s][b,o+1,v] * (target[b,v] == o+1)
    p2[s,b,o]    = sum_v pred[s][b,o+1,v]^2
    t2[b,o]      = sum_v (target[b,v] == o+1)
    dice[b]      = sum_{s,o} 2*inter/(p2+t2+eps) / 13
    loss         = mean_b (2 - dice[b])

Device strategy: the flat spatial volume (48*256*256 voxels) is split
contiguously 8 ways; each core streams its 1/8 of both preds (channels
1..13 only) and of the target.  Per (b,o): one DVE tensor_scalar
(is_equal, accum_out) builds the one-hot mask and its count t2.  Per
(s,b,o): one DVE tensor_tensor(mult) forms the masked product (bf16),
which the TensorEngine reduces with ones-vector matmuls accumulated in a
PSUM row, extracted by a ScalarE Copy+accum; p^2 is one ScalarE
activation(Square, accum_out).  Per-partition partials are DMAed out;
the final tiny reduction and dice formula run on host in float64.
"""

import contextlib

import numpy as np

import concourse.bacc as bacc
import concourse.tile as tile
from concourse import mybir
from concourse.bass_utils import run_bass_kernel_spmd

N_CORES = 8
B = 2
NUM_ORGAN = 13
VOX = 48 * 256 * 256          # 3,145,728 voxels per (b, organ) volume
SHARD = VOX // N_CORES        # 393,216 voxels per core
P = 128                       # SBUF partitions
FD = SHARD // P               # 3072 free-dim elements per partition
MM_N = 512                    # matmul free-dim chunk (one PSUM bank)
EPS = 1e-05

_NC_CACHE = {}


def build_nc(loop_k=None):
    f32 = mybir.dt.float32
    bf16 = mybir.dt.bfloat16
    nc = bacc.Bacc(
        "TRN2",
        target_bir_lowering=False,
        debug=False,
        num_devices=N_CORES,
    )
    p1 = nc.dram_tensor("p1", [B, NUM_ORGAN, P, FD], f32, kind="ExternalInput").ap()
    p2 = nc.dram_tensor("p2", [B, NUM_ORGAN, P, FD], f32, kind="ExternalInput").ap()
    tt = nc.dram_tensor("tt", [B, P, FD], f32, kind="ExternalInput").ap()
    out_t2 = nc.dram_tensor(
        "out_t2", [P, B * NUM_ORGAN], f32, kind="ExternalOutput"
    ).ap()
    out_inter = nc.dram_tensor(
        "out_inter", [1, 2 * B * NUM_ORGAN], f32, kind="ExternalOutput"
    ).ap()
    out_p2 = nc.dram_tensor(
        "out_p2", [P, 2 * B * NUM_ORGAN], f32, kind="ExternalOutput"
    ).ap()

    with tile.TileContext(nc) as tc, \
            tc.tile_pool(name="pin", bufs=4) as pin_pool, \
            tc.tile_pool(name="tin", bufs=2) as t_pool, \
            tc.tile_pool(name="mask", bufs=2) as mask_pool, \
            tc.tile_pool(name="prod", bufs=2) as prod_pool, \
            tc.tile_pool(name="scr", bufs=2) as scr_pool, \
            tc.tile_pool(name="psum", bufs=6, space="PSUM") as psum_pool, \
            tc.tile_pool(name="acc", bufs=1) as acc_pool:
        acc_t2 = acc_pool.tile([P, B * NUM_ORGAN], f32, tag="acc_t2")
        acc_in = acc_pool.tile([1, 2 * B * NUM_ORGAN], f32, tag="acc_in")
        acc_p2 = acc_pool.tile([P, 2 * B * NUM_ORGAN], f32, tag="acc_p2")
        ones = acc_pool.tile([P, 1], bf16, tag="ones")
        nc.vector.memset(ones[:], 1.0)
        for _ in range(loop_k or 1):
            _emit_body(nc, tc, p1, p2, tt, pin_pool, t_pool, mask_pool,
                       prod_pool, scr_pool, psum_pool, ones,
                       acc_t2, acc_in, acc_p2)
        nc.sync.dma_start(out_t2[:], acc_t2[:])
        nc.sync.dma_start(out_inter[:], acc_in[:])
        nc.sync.dma_start(out_p2[:], acc_p2[:])
    nc.compile()
    return nc


def _emit_body(nc, tc, p1, p2, tt, pin_pool, t_pool, mask_pool, prod_pool,
               scr_pool, psum_pool, ones, acc_t2, acc_in, acc_p2):
    f32 = mybir.dt.float32
    bf16 = mybir.dt.bfloat16
    if True:
        for b in range(B):
            t_tile = t_pool.tile([P, FD], f32, tag="t")
            nc.sync.dma_start(t_tile[:], tt[b])
            for o in range(NUM_ORGAN):
                col = b * NUM_ORGAN + o
                mask = mask_pool.tile([P, FD], f32, tag="m")
                nc.vector.tensor_scalar(
                    mask[:],
                    t_tile[:],
                    float(o + 1),
                    None,
                    op0=mybir.AluOpType.is_equal,
                    op1=mybir.AluOpType.add,
                    accum_out=acc_t2[:, col:col + 1],
                )
                for s, pin in enumerate((p1, p2)):
                    col2 = col * 2 + s
                    pt = pin_pool.tile([P, FD], f32, tag="p")
                    nc.sync.dma_start(pt[:], pin[b, o])
                    # p^2 partial sums on ScalarE
                    scr_a = scr_pool.tile([P, FD], bf16, tag="sa")
                    nc.scalar.activation(
                        out=scr_a[:],
                        in_=pt[:],
                        func=mybir.ActivationFunctionType.Square,
                        accum_out=acc_p2[:, col2:col2 + 1],
                    )
                    # masked product on DVE (bf16 out), reduced on PE
                    prod = prod_pool.tile([P, FD], bf16, tag="pr")
                    nc.vector.tensor_tensor(
                        prod[:], pt[:], mask[:], op=mybir.AluOpType.mult
                    )
                    ps = psum_pool.tile([1, MM_N], f32, tag="ps")
                    nchunk = FD // MM_N
                    for j in range(nchunk):
                        nc.tensor.matmul(
                            ps[:],
                            ones[:],
                            prod[:, j * MM_N:(j + 1) * MM_N],
                            start=(j == 0),
                            stop=(j == nchunk - 1),
                        )
                    scr_e = scr_pool.tile([1, MM_N], f32, tag="se")
                    nc.scalar.activation(
                        out=scr_e[:],
                        in_=ps[:],
                        func=mybir.ActivationFunctionType.Copy,
                        accum_out=acc_in[:, col2:col2 + 1],
                    )


def _get_nc():
    if "nc" not in _NC_CACHE:
        _NC_CACHE["nc"] = build_nc()
    return _NC_CACHE["nc"]


def make_in_maps(pred_stage1, pred_stage2, target):
    """Shard full inputs into per-core input maps (host-side)."""
    p1 = np.asarray(pred_stage1)[:, 1:1 + NUM_ORGAN].reshape(B, NUM_ORGAN, VOX)
    p2 = np.asarray(pred_stage2)[:, 1:1 + NUM_ORGAN].reshape(B, NUM_ORGAN, VOX)
    tf = np.asarray(target).astype(np.float32).reshape(B, VOX)
    in_maps = []
    for c in range(N_CORES):
        sl = slice(c * SHARD, (c + 1) * SHARD)
        in_maps.append({
            "p1": np.ascontiguousarray(p1[:, :, sl]).reshape(B, NUM_ORGAN, P, FD),
            "p2": np.ascontiguousarray(p2[:, :, sl]).reshape(B, NUM_ORGAN, P, FD),
            "tt": np.ascontiguousarray(tf[:, sl]).reshape(B, P, FD),
        })
    return in_maps


def finalize(results):
    """Combine per-core partials into the scalar loss (host-side)."""
    t2 = np.zeros(B * NUM_ORGAN, np.float64)
    inter = np.zeros(2 * B * NUM_ORGAN, np.float64)
    p2s = np.zeros(2 * B * NUM_ORGAN, np.float64)
    for r in results:
        t2 += r["out_t2"].astype(np.float64).sum(axis=0)
        inter += r["out_inter"].astype(np.float64).sum(axis=0)
        p2s += r["out_p2"].astype(np.float64).sum(axis=0)
    t2 = t2.reshape(B, NUM_ORGAN)
    inter = inter.reshape(B, NUM_ORGAN, 2)
    p2s = p2s.reshape(B, NUM_ORGAN, 2)
    dice = 2.0 * inter / (p2s + t2[:, :, None] + EPS)   # (B, 13, 2)
    dice_b = dice.sum(axis=(1, 2)) / NUM_ORGAN          # (B,)
    loss = np.mean(2.0 - dice_b)
    return np.array(loss, dtype=np.float32)


def kernel(pred_stage1, pred_stage2, target):
    nc = _get_nc()
    in_maps = make_in_maps(pred_stage1, pred_stage2, target)
    res = run_bass_kernel_spmd(nc, in_maps, core_ids=list(range(N_CORES)))
    return finalize(res.results)


# revision 33
# speedup vs baseline: 4.4547x; 1.1193x over previous
"""Dice-loss kernel for Trainium2 (Bass/Tile), 8-way data parallel.

Math (per stage s, batch b, organ o; organ ids 1..13):
    inter[s,b,o] = sum_v pred[s][b,o,v] * (target[b,v] == o)
    p2[s,b,o]    = sum_v pred[s][b,o,v]^2
    t2[b,o]      = sum_v (target[b,v] == o)
    dice[b]      = sum_{s,o} 2*inter/(p2+t2+eps) / 13
    loss         = mean_b (2 - dice[b])

Device strategy: the flat spatial volume (48*256*256 voxels) is split
contiguously 8 ways; each core streams its 1/8 of both preds (channels
1..13 only, cast to fp16 on host - exact to ~2^-11 for uniform [0,1)
data - halving HBM traffic) and of the target (cast to fp16, exact for
labels 0..14).  Per (s,b,o) slice one DVE scalar_tensor_tensor computes
out=(t==o)*p with accum_out = its sum = inter (2x-mode, 16-bit);
likewise one stt against a ones tile yields t2; p^2 comes from one
ScalarE activation(Square, accum_out).  Per-partition fp32 partials
[128, col] are DMAed out; the final tiny reduction and the dice formula
run on host in float64.

Mode string flags (combined, e.g. "f16t+not2+sp10"):
    f16t  - ship target as f16 and run 16-bit stt ops
    not2  - skip device t2; host computes it via bincount
    spN   - N of the 52 squares run on DVE instead of ScalarE
    dma   - DMA-only ablation (for benchmarking)
"""

import numpy as np

import concourse.bacc as bacc
import concourse.tile as tile
from concourse import mybir
from concourse.bass_utils import run_bass_kernel_spmd

N_CORES = 8
B = 2
NUM_ORGAN = 13
VOX = 48 * 256 * 256          # 3,145,728 voxels per (b, organ) volume
SHARD = VOX // N_CORES        # 393,216 voxels per core
P = 128                       # SBUF partitions
FD = SHARD // P               # 3072 free-dim elements per partition
EPS = 1e-05

DESIGN = "f16t+not2+sn4"      # the mode kernel() uses

_NC_CACHE = {}


def _flags(mode):
    toks = mode.split("+")
    sp = 0
    spm = 0
    sn = 1
    for t in toks:
        if t.startswith("spm"):
            spm = int(t[3:])
        elif t == "split2":
            sn = 2
        elif t.startswith("sn"):
            sn = int(t[2:])
        elif t.startswith("sp"):
            sp = int(t[2:])
    return {
        "dma": "dma" in toks,
        "f16t": "f16t" in toks,
        "not2": "not2" in toks,
        "nosq": "nosq" in toks,
        "sn": sn,
        "sp": sp,
        "spm": spm,
    }


def build_nc(loop_k=None, mode=DESIGN):
    fl = _flags(mode)
    f32 = mybir.dt.float32
    f16 = mybir.dt.float16
    bf16 = mybir.dt.bfloat16
    tdt = f16 if fl["f16t"] else f32
    nc = bacc.Bacc(
        "TRN2",
        target_bir_lowering=False,
        debug=False,
        num_devices=N_CORES,
    )
    p1 = nc.dram_tensor("p1", [B, NUM_ORGAN, P, FD], f16, kind="ExternalInput").ap()
    p2 = nc.dram_tensor("p2", [B, NUM_ORGAN, P, FD], f16, kind="ExternalInput").ap()
    tt = nc.dram_tensor("tt", [B, P, FD], tdt, kind="ExternalInput").ap()
    out_t2 = nc.dram_tensor(
        "out_t2", [P, B * NUM_ORGAN], f32, kind="ExternalOutput"
    ).ap()
    out_inter = nc.dram_tensor(
        "out_inter", [P, 2 * B * NUM_ORGAN], f32, kind="ExternalOutput"
    ).ap()
    out_p2 = nc.dram_tensor(
        "out_p2", [P, 2 * B * NUM_ORGAN], f32, kind="ExternalOutput"
    ).ap()

    with tile.TileContext(nc) as tc, \
            tc.tile_pool(name="pin", bufs=4) as pin_pool, \
            tc.tile_pool(name="tin", bufs=2) as t_pool, \
            tc.tile_pool(name="mask", bufs=2) as mask_pool, \
            tc.tile_pool(name="prod", bufs=2) as prod_pool, \
            tc.tile_pool(name="scr", bufs=2) as scr_pool, \
            tc.tile_pool(name="acc", bufs=1) as acc_pool:
        acc_t2 = acc_pool.tile([P, B * NUM_ORGAN], f32, tag="acc_t2")
        acc_in = acc_pool.tile([P, 2 * B * NUM_ORGAN], f32, tag="acc_in")
        acc_p2 = acc_pool.tile([P, 2 * B * NUM_ORGAN], f32, tag="acc_p2")
        ones = acc_pool.tile([P, FD], tdt, tag="ones")
        nc.vector.memset(acc_t2[:], 0.0)
        nc.vector.memset(acc_in[:], 0.0)
        nc.vector.memset(acc_p2[:], 0.0)
        nc.vector.memset(ones[:], 1.0)
        for _ in range(loop_k or 1):
            _emit_body(nc, fl, p1, p2, tt, pin_pool, t_pool, mask_pool,
                       prod_pool, scr_pool, ones, acc_t2, acc_in, acc_p2)
        nc.sync.dma_start(out_t2[:], acc_t2[:])
        nc.sync.dma_start(out_inter[:], acc_in[:])
        nc.sync.dma_start(out_p2[:], acc_p2[:])
    nc.compile()
    return nc


def _emit_body(nc, fl, p1, p2, tt, pin_pool, t_pool, mask_pool, prod_pool,
               scr_pool, ones, acc_t2, acc_in, acc_p2):
    f32 = mybir.dt.float32
    f16 = mybir.dt.float16
    bf16 = mybir.dt.bfloat16
    tdt = f16 if fl["f16t"] else f32
    n_sq_dve = fl["sp"]
    for b in range(B):
        t_tile = t_pool.tile([P, FD], tdt, tag="t")
        nc.sync.dma_start(t_tile[:], tt[b])
        if fl["dma"]:
            for o in range(NUM_ORGAN):
                for s, pin in enumerate((p1, p2)):
                    pt = pin_pool.tile([P, FD], f16, tag="p")
                    nc.sync.dma_start(pt[:], pin[b, o])
            continue
        for o in range(NUM_ORGAN):
            col = b * NUM_ORGAN + o
            if not fl["not2"]:
                # t2 on DVE: sum((t == o) * 1)
                scr_m = mask_pool.tile([P, FD], tdt, tag="m")
                nc.vector.scalar_tensor_tensor(
                    scr_m[:],
                    t_tile[:],
                    float(o + 1),
                    ones[:],
                    op0=mybir.AluOpType.is_equal,
                    op1=mybir.AluOpType.mult,
                    accum_out=acc_t2[:, col:col + 1],
                )
            for s, pin in enumerate((p1, p2)):
                col2 = col * 2 + s
                pt = pin_pool.tile([P, FD], f16, tag="p")
                nc.sync.dma_start(pt[:], pin[b, o])
                # inter on DVE: accum_out = sum((t == o) * p)
                scr_v = prod_pool.tile([P, FD], tdt, tag="pr")
                sn = fl["sn"]
                if sn > 1:
                    h = FD // sn
                    acc_b = mask_pool.tile([P, sn], mybir.dt.float32, tag="ab")
                    for hh in range(sn):
                        nc.vector.scalar_tensor_tensor(
                            scr_v[:, hh * h:(hh + 1) * h],
                            t_tile[:, hh * h:(hh + 1) * h],
                            float(o + 1),
                            pt[:, hh * h:(hh + 1) * h],
                            op0=mybir.AluOpType.is_equal,
                            op1=mybir.AluOpType.mult,
                            accum_out=acc_b[:, hh:hh + 1],
                        )
                    nc.vector.tensor_reduce(
                        acc_in[:, col2:col2 + 1], acc_b[:],
                        axis=mybir.AxisListType.X, op=mybir.AluOpType.add,
                    )
                else:
                    nc.vector.scalar_tensor_tensor(
                        scr_v[:],
                        t_tile[:],
                        float(o + 1),
                        pt[:],
                        op0=mybir.AluOpType.is_equal,
                        op1=mybir.AluOpType.mult,
                        accum_out=acc_in[:, col2:col2 + 1],
                    )
                if fl["nosq"]:
                    continue
                if col2 < n_sq_dve or (fl["spm"] and col2 % fl["spm"] == 0):
                    # p^2 on DVE: sum((p bypass) * p)
                    scr_d = scr_pool.tile([P, FD], f16, tag="sd")
                    nc.vector.scalar_tensor_tensor(
                        scr_d[:],
                        pt[:],
                        0.0,
                        pt[:],
                        op0=mybir.AluOpType.bypass,
                        op1=mybir.AluOpType.mult,
                        accum_out=acc_p2[:, col2:col2 + 1],
                    )
                else:
                    # p^2 on ScalarE
                    scr_a = scr_pool.tile([P, FD], bf16, tag="sa")
                    nc.scalar.activation(
                        out=scr_a[:],
                        in_=pt[:],
                        func=mybir.ActivationFunctionType.Square,
                        accum_out=acc_p2[:, col2:col2 + 1],
                    )


def _get_nc():
    if "nc" not in _NC_CACHE:
        _NC_CACHE["nc"] = build_nc()
    return _NC_CACHE["nc"]


def make_in_maps(pred_stage1, pred_stage2, target, mode=DESIGN):
    """Shard full inputs into per-core input maps (host-side)."""
    fl = _flags(mode)
    tdt = np.float16 if fl["f16t"] else np.float32
    p1 = np.asarray(pred_stage1)[:, 1:1 + NUM_ORGAN].reshape(B, NUM_ORGAN, VOX)
    p2 = np.asarray(pred_stage2)[:, 1:1 + NUM_ORGAN].reshape(B, NUM_ORGAN, VOX)
    p1 = p1.astype(np.float16)
    p2 = p2.astype(np.float16)
    tf = np.asarray(target).astype(tdt).reshape(B, VOX)
    in_maps = []
    for c in range(N_CORES):
        sl = slice(c * SHARD, (c + 1) * SHARD)
        in_maps.append({
            "p1": np.ascontiguousarray(p1[:, :, sl]).reshape(B, NUM_ORGAN, P, FD),
            "p2": np.ascontiguousarray(p2[:, :, sl]).reshape(B, NUM_ORGAN, P, FD),
            "tt": np.ascontiguousarray(tf[:, sl]).reshape(B, P, FD),
        })
    return in_maps


def finalize(results, target=None, mode=DESIGN):
    """Combine per-core partials into the scalar loss (host-side)."""
    fl = _flags(mode)
    t2 = np.zeros(B * NUM_ORGAN, np.float64)
    inter = np.zeros(2 * B * NUM_ORGAN, np.float64)
    p2s = np.zeros(2 * B * NUM_ORGAN, np.float64)
    for r in results:
        t2 += r["out_t2"].astype(np.float64).sum(axis=0)
        inter += r["out_inter"].astype(np.float64).sum(axis=0)
        p2s += r["out_p2"].astype(np.float64).sum(axis=0)
    t2 = t2.reshape(B, NUM_ORGAN)
    if fl["not2"]:
        assert target is not None
        tt = np.asarray(target).reshape(B, VOX)
        t2 = np.stack([
            np.bincount(tt[b], minlength=NUM_ORGAN + 1)[1:NUM_ORGAN + 1]
            for b in range(B)
        ]).astype(np.float64)
    inter = inter.reshape(B, NUM_ORGAN, 2)
    p2s = p2s.reshape(B, NUM_ORGAN, 2)
    dice = 2.0 * inter / (p2s + t2[:, :, None] + EPS)   # (B, 13, 2)
    dice_b = dice.sum(axis=(1, 2)) / NUM_ORGAN          # (B,)
    loss = np.mean(2.0 - dice_b)
    return np.array(loss, dtype=np.float32)


def kernel(pred_stage1, pred_stage2, target):
    nc = _get_nc()
    in_maps = make_in_maps(pred_stage1, pred_stage2, target)
    last_err = None
    for _ in range(3):   # retry transient device/transport errors
        try:
            res = run_bass_kernel_spmd(nc, in_maps, core_ids=list(range(N_CORES)))
            return finalize(res.results, target=target)
        except Exception as e:   # noqa: BLE001
            last_err = e
    raise last_err
